# revision 1
# baseline (speedup 1.0000x reference)
"""GCN layer kernel for Trainium2, data-parallel over batch on 8 NeuronCores.

Math per batch b (N=2048, F=256):
    r[n]  = sum_k adj[n, k];  d = (r + 1)^-1/2          (adj_hat = adj + I)
    X'    = d * X   (row scale)
    Hhat  = d * (adj @ X' + X')   = D^-1/2 (adj+I) D^-1/2 X
    out   = relu(Hhat @ W.T + b)

Device computes everything in transposed form (out^T = [256, 2048]) so both
matmuls contract on the partition axis with zero on-device transposes of adj:
the host pre-permutes adj into 4 row-strips laid out so that each strip is a
contiguous 4 MB DMA delivering ready-to-use [k, i] blocks:
    adjs[c, p, t, jj] = adj[512 c + jj, 128 t + p]
Strip c holds full adj rows 512c..512c+511, so its rowsums (and d for those
nodes) complete from that strip alone -> matmul work overlaps the adj stream.

H^T chunk (fb, ic) accumulates over t in 4 quarters; quarter q of chunk
(fb, ic) only needs strip ic (rhs) and strip q (d -> X'), so the growing
set of arrived strips steadily unlocks matmul work.
"""

import sys
import types
import numpy as np

for _p in ("/root/.axon_site/_ro/trn_rl_repo", "/opt/trn_rl_repo"):
    if _p not in sys.path:
        sys.path.append(_p)

import concourse.bacc as bacc
import concourse.mybir as mybir
import concourse.tile as tile
from concourse.bass_utils import run_bass_kernel_spmd
from concourse.masks import make_identity

N_CORES = 8
N = 2048        # nodes
F = 256         # in/out feature dim
NSTRIP = 4      # adj row-strips per core
SW = N // NSTRIP          # 512 rows per strip
NT = N // 128             # 16 k-tiles
NQ = 4                    # t-quarters per H^T chunk
F32 = mybir.dt.float32
F32R = mybir.dt.float32r
BF16 = mybir.dt.bfloat16
USE_BF16 = True
MMDT = BF16 if USE_BF16 else F32R
AF = mybir.ActivationFunctionType


def _install_axon_hooks():
    """The image's `antenv` lacks `axon_hooks`, which concourse imports for
    trace=True under axon. Install a minimal get/set shim and register the
    NTFF profile hook so traces (HW exec time) work."""
    if "antenv.axon_hooks" in sys.modules:
        return
    mod = types.ModuleType("antenv.axon_hooks")
    _hook = [None]
    mod.set_axon_ntff_profile_hook = lambda h: _hook.__setitem__(0, h)
    mod.get_axon_ntff_profile_hook = lambda: _hook[0]
    sys.modules["antenv.axon_hooks"] = mod
    import antenv
    antenv.axon_hooks = mod
    try:
        from trn_agent_boot.trn_boot import _ntff_profile_via_ctypes
        mod.set_axon_ntff_profile_hook(
            _ntff_profile_via_ctypes("/opt/axon/libaxon_pjrt.so")
        )
    except Exception:
        pass


def _emit(nc, tc, pools, adjs, xdram, wtdram, bdram, outT):
    sb, ps_mm1, ps_small, ps_mm2 = (
        pools["sb"], pools["ps_mm1"], pools["ps_small"], pools["ps_mm2"])
    consts = pools["consts"]

    # ---- constants ----
    ident = consts.tile([128, 128], F32)
    make_identity(nc, ident)
    eye_r = consts.tile([128, 128], MMDT)
    nc.scalar.activation(eye_r, ident, AF.Copy)
    ones_f = consts.tile([128, 128], F32)
    nc.vector.memset(ones_f, 1.0)
    ones_mat = consts.tile([128, 128], MMDT)
    nc.scalar.activation(ones_mat, ones_f, AF.Copy)

    # Warm-up matmuls: PE sits idle until the first strip lands; HAM then
    # runs the first real matmul bursts at the throttled 1.2 GHz clock.
    # Burn the idle head on dependency-free matmuls to hold K=8/8.
    wu = ps_mm2.tile([128, 512], F32, tag="mm2")
    for i in range(80):
        nc.tensor.matmul(wu[:, 0:128], ones_mat, eye_r,
                         start=(i == 0), stop=(i == 79))
    wusb = consts.tile([128, 1], F32)
    nc.scalar.activation(wusb, wu[:, 0:1], AF.Copy)

    # ---- small persistent tensors ----
    wt_t = []
    for ft in range(2):
        w = consts.tile([128, F], MMDT, tag=f"wt{ft}")
        nc.sync.dma_start(out=w, in_=wtdram[128 * ft:128 * (ft + 1), :])
        wt_t.append(w)
    brow = consts.tile([1, F], F32)
    nc.scalar.dma_start(out=brow, in_=bdram[:, :])
    b_col = []
    for ob in range(2):
        pb = ps_small.tile([128, 1], F32, tag="small")
        nc.tensor.transpose(pb, brow[:, 128 * ob:128 * (ob + 1)], ident[0:1, 0:1])
        bc = consts.tile([128, 1], F32, tag=f"bcol{ob}")
        nc.scalar.activation(bc, pb, AF.Copy)
        b_col.append(bc)

    strips = [None] * NSTRIP
    x_t = [None] * NT
    xp_t = [None] * NT
    rep_d = [None] * NSTRIP   # per-chunk d, replicated across partitions
    acc = {}
    hT = {}

    def emit_dmas(c):
        # Emit every input DMA up front so the two HWDGE rings and the
        # SWDGE queue drain back-to-back: interleaving the triggers with
        # compute on the issuing engines left the DMA idle 60% of the time.
        # Each strip is split across both rings so strips complete strictly
        # one after another, unlocking rowsums/matmul work early.
        st = sb.tile([128, NT * SW], MMDT, tag="strip", bufs=NSTRIP)
        strips[c] = st
        half = NT * SW // 2
        nc.sync.dma_start(out=st[:, 0:half], in_=adjs[c][:, 0:half])
        nc.scalar.dma_start(out=st[:, half:], in_=adjs[c][:, half:])
        # X rows for the k-blocks whose d comes from this strip: the host
        # lays X out per-strip partition-major so this is one contiguous
        # SWDGE transfer that stays off the two HWDGE rings.
        xc = sb.tile([128, 4, F], MMDT, tag="x", bufs=NSTRIP)
        nc.gpsimd.dma_start(out=xc, in_=xdram[c].rearrange(
            "p (h f) -> p h f", h=4))
        for h in range(4):
            x_t[4 * c + h] = xc[:, h, :]

    def emit_strip(c):
        st = strips[c]
        # rowsum of adj rows [512c, 512c+512): all-ones.T @ strip block
        # accumulated over the 16 t-blocks in PSUM; every psum partition
        # holds the same column-sums, so d comes out already replicated.
        pr = ps_small.tile([128, SW], F32, tag="small")
        for t in range(NT):
            nc.tensor.matmul(
                pr, ones_mat, st[:, SW * t:SW * (t + 1)],
                start=(t == 0), stop=(t == NT - 1))
        s_chunk = sb.tile([128, SW], F32, tag="schunk", bufs=2)
        nc.scalar.activation(s_chunk, pr, AF.Sqrt, bias=1.0)   # sqrt(r + 1)
        for h in range(4):
            t = 4 * c + h
            # rows of s_chunk are identical, so transposing a [128, 128]
            # slice yields sqrt(r+1) for k-block t in column form; a [128,1]
            # reciprocal is ~20x cheaper than a [128,512] one, keeping the
            # d -> X' chain short.
            pdc = ps_small.tile([128, 128], F32, tag="small")
            nc.tensor.transpose(pdc, s_chunk[:, 128 * h:128 * (h + 1)], ident)
            scol = sb.tile([128, 1], F32, tag="scol", bufs=8)
            nc.scalar.activation(scol, pdc[:, 0:1], AF.Copy)
            dcol = sb.tile([128, 1], F32, tag="dcol", bufs=8)
            nc.vector.reciprocal(dcol, scol)
            xp = sb.tile([128, F], MMDT, tag="xp", bufs=NT)
            nc.scalar.activation(xp, x_t[t], AF.Copy, scale=dcol)
            xp_t[t] = xp
        # wide reciprocal for the H^T row-scale, off the critical chain
        rd = sb.tile([128, SW], F32, tag="repd", bufs=NSTRIP)
        nc.vector.reciprocal(rd, s_chunk)
        rep_d[c] = rd

    def emit_group(fb, ic, q):
        """Accumulate quarter q (t in [4q, 4q+4)) of H^T chunk (fb, ic)."""
        P = ps_mm1.tile([128, 512], F32, tag="mm1")
        n_mm = 4 + (4 if q == ic else 0)
        i_mm = 0
        for t in range(4 * q, 4 * q + 4):
            nc.tensor.matmul(
                P,
                xp_t[t][:, 128 * fb:128 * (fb + 1)],
                strips[ic][:, SW * t:SW * (t + 1)],
                start=(i_mm == 0), stop=(i_mm == n_mm - 1))
            i_mm += 1
        if q == ic:
            # identity fold: adds X'^T into columns 128t..128t+128
            for t in range(4 * q, 4 * q + 4):
                lo = 128 * (t - 4 * q)
                nc.tensor.matmul(
                    P[:, lo:lo + 128],
                    xp_t[t][:, 128 * fb:128 * (fb + 1)],
                    eye_r,
                    start=False, stop=(i_mm == n_mm - 1))
                i_mm += 1
        key = (fb, ic)
        if q == 0:
            a = sb.tile([128, 512], F32, tag="acc", bufs=8)
            nc.scalar.activation(a, P, AF.Copy)
            acc[key] = a
        else:
            nc.vector.tensor_add(acc[key], acc[key], P)
        if q == NQ - 1:
            h = sb.tile([128, 512], MMDT, tag="hT", bufs=4)
            nc.vector.tensor_mul(h, acc[key], rep_d[ic])
            hT[key] = h

    def emit_mm2(ic):
        for ob in range(2):
            P2 = ps_mm2.tile([128, 512], F32, tag="mm2")
            for fb in range(2):
                nc.tensor.matmul(
                    P2, wt_t[fb][:, 128 * ob:128 * (ob + 1)], hT[(fb, ic)],
                    start=(fb == 0), stop=(fb == 1))
            o = sb.tile([128, 512], F32, tag="osb", bufs=4)
            nc.scalar.activation(o, P2, AF.Relu, bias=b_col[ob])
            nc.sync.dma_start(
                out=outT[128 * ob:128 * (ob + 1), 512 * ic:512 * (ic + 1)],
                in_=o)

    # group (fb, ic, q) is ready once strips max(ic, q) .. 0 have arrived
    by_strip = {s: [] for s in range(NSTRIP)}
    for ic in range(NSTRIP):
        for q in range(NQ):
            s = max(ic, q)
            for fb in range(2):
                by_strip[s].append((fb, ic, q))

    for c in range(NSTRIP):
        emit_dmas(c)
    for c in range(NSTRIP):
        emit_strip(c)
        for (fb, ic, q) in sorted(by_strip[c], key=lambda g: (g[2], g[1], g[0])):
            emit_group(fb, ic, q)
            if q == NQ - 1 and (1, ic) in hT and (0, ic) in hT:
                emit_mm2(ic)


_CACHE = {}


def _build():
    if "nc" in _CACHE:
        return _CACHE["nc"]
    _install_axon_hooks()
    nc = bacc.Bacc("TRN2", target_bir_lowering=False, debug=False,
                   num_devices=N_CORES)
    adjs = [nc.dram_tensor(f"adjs{c}", [128, NT * SW], MMDT,
                           kind="ExternalInput").ap()
            for c in range(NSTRIP)]
    xdram = [nc.dram_tensor(f"x{c}", [128, 4 * F], MMDT,
                            kind="ExternalInput").ap()
             for c in range(NSTRIP)]
    wtdram = nc.dram_tensor("wt", [F, F], MMDT, kind="ExternalInput").ap()
    bdram = nc.dram_tensor("b", [1, F], F32, kind="ExternalInput").ap()
    outT = nc.dram_tensor("outT", [F, N], F32, kind="ExternalOutput").ap()

    with tile.TileContext(nc) as tc:
        with tc.tile_pool(name="consts", bufs=1) as consts, \
             tc.tile_pool(name="sb", bufs=2) as sb, \
             tc.tile_pool(name="ps_mm1", bufs=3, space="PSUM") as ps_mm1, \
             tc.tile_pool(name="ps_small", bufs=3, space="PSUM") as ps_small, \
             tc.tile_pool(name="ps_mm2", bufs=2, space="PSUM") as ps_mm2:
            pools = dict(consts=consts, sb=sb, ps_mm1=ps_mm1,
                         ps_small=ps_small, ps_mm2=ps_mm2)
            _emit(nc, tc, pools, adjs, xdram, wtdram, bdram, outT)
    nc.compile()
    _CACHE["nc"] = nc
    return nc


def _shard(inputs):
    X = np.ascontiguousarray(np.asarray(inputs["X"], dtype=np.float32))
    adj = np.asarray(inputs["adj"], dtype=np.float32)
    W = np.asarray(inputs["W"], dtype=np.float32)
    b = np.asarray(inputs["b"], dtype=np.float32)
    np_mmdt = mybir.dt.np(MMDT)
    wt = np.ascontiguousarray(W.T).astype(np_mmdt)
    brow = b.reshape(1, F)
    in_maps = []
    for c in range(N_CORES):
        # adjs[s, p, t, jj] = adj[c][512 s + jj, 128 t + p]
        a4 = adj[c].reshape(NSTRIP, SW, NT, 128)
        strips = np.ascontiguousarray(a4.transpose(0, 3, 2, 1)).reshape(
            NSTRIP, 128, NT * SW).astype(np_mmdt)
        m = {f"adjs{s}": strips[s] for s in range(NSTRIP)}
        xs = X[c].reshape(NSTRIP, 4, 128, F).transpose(0, 2, 1, 3)
        xs = np.ascontiguousarray(xs).reshape(NSTRIP, 128, 4 * F).astype(np_mmdt)
        for s in range(NSTRIP):
            m[f"x{s}"] = xs[s]
        m["wt"] = wt
        m["b"] = brow
        in_maps.append(m)
    return in_maps


def run(inputs, trace=False):
    nc = _build()
    in_maps = _shard(inputs)
    res = run_bass_kernel_spmd(
        nc, in_maps, core_ids=list(range(N_CORES)), trace=trace)
    out = np.stack([r["outT"].T for r in res.results]).astype(np.float32)
    return np.ascontiguousarray(out), res


def kernel(**inputs):
    return run(inputs, trace=False)[0]



# revision 8
# speedup vs baseline: 1.0350x; 1.0350x over previous
"""GCN layer kernel for Trainium2, data-parallel over batch on 8 NeuronCores.

Math per batch b (N=2048, F=256):
    r[n]  = sum_k adj[n, k];  d = (r + 1)^-1/2          (adj_hat = adj + I)
    X'    = d * X   (row scale)
    Hhat  = d * (adj @ X' + X')   = D^-1/2 (adj+I) D^-1/2 X
    out   = relu(Hhat @ W.T + b)

Device computes everything in transposed form (out^T = [256, 2048]) so both
matmuls contract on the partition axis with zero on-device transposes of adj.
adj is quantized to fp8 e4m3 on the host, which both halves the adj DMA
stream and enables MatmulPerfMode.DoubleRow (K=256 per instruction at 0.5
cycles/row — 4x the bf16 matmul rate). X' = S*d*X is quantized on-device to
fp8 as a hi value plus a same-scale fp8 residual; both stationary sets
accumulate into the same PSUM group, so the X-side quantization error drops
~2x for only a second LDWEIGHTS+matmul per tile pair (adj stream is reused).

Host pre-permutes adj into 4 row-strips of ready-to-use [k, i] blocks:
    adjs[c][p, t, jj] = adj[512 c + jj, 128 t + p]
Strip c holds full adj rows 512c..512c+511, so its rowsums (and d for those
nodes) complete from that strip alone -> matmul work overlaps the adj stream.

H^T chunk (fb, ic) accumulates over 4 segments (one per k-strip); segment s
of chunk (fb, ic) only needs strip ic (rhs) and strip s (d -> X'), so the
growing set of arrived strips steadily unlocks matmul work, and chunks with
ic < 3 finish their mm2 while the ic=3 segments still run.
"""

import sys
import types
import numpy as np

for _p in ("/root/.axon_site/_ro/trn_rl_repo", "/opt/trn_rl_repo"):
    if _p not in sys.path:
        sys.path.append(_p)

import concourse.bacc as bacc
import concourse.mybir as mybir
import concourse.tile as tile
from concourse.bass_utils import run_bass_kernel_spmd
from concourse.masks import make_identity

N_CORES = 8
N = 2048        # nodes
F = 256         # in/out feature dim
NSTRIP = 4      # adj row-strips per core
SW = N // NSTRIP          # 512 rows per strip
NT = N // 128             # 16 k-tiles
NPAIR = NT // 2           # 8 DoubleRow k-pairs
F32 = mybir.dt.float32
BF16 = mybir.dt.bfloat16
FP8 = mybir.dt.float8e4
AF = mybir.ActivationFunctionType
DR = mybir.MatmulPerfMode.DoubleRow
XSPLIT = True             # add same-scale fp8 residual pass for X'
S = 16.0                  # fp8 scale for X' (keeps values out of subnormals)
S2 = S * S


def _install_axon_hooks():
    """The image's `antenv` lacks `axon_hooks`, which concourse imports for
    trace=True under axon. Install a minimal get/set shim and register the
    NTFF profile hook so traces (HW exec time) work."""
    if "antenv.axon_hooks" in sys.modules:
        return
    mod = types.ModuleType("antenv.axon_hooks")
    _hook = [None]
    mod.set_axon_ntff_profile_hook = lambda h: _hook.__setitem__(0, h)
    mod.get_axon_ntff_profile_hook = lambda: _hook[0]
    sys.modules["antenv.axon_hooks"] = mod
    import antenv
    antenv.axon_hooks = mod
    try:
        from trn_agent_boot.trn_boot import _ntff_profile_via_ctypes
        mod.set_axon_ntff_profile_hook(
            _ntff_profile_via_ctypes("/opt/axon/libaxon_pjrt.so")
        )
    except Exception:
        pass


def _emit(nc, tc, pools, adjs, xdram, wtdram, bdram, outT):
    sb, ps_mm1, ps_small, ps_mm2 = (
        pools["sb"], pools["ps_mm1"], pools["ps_small"], pools["ps_mm2"])
    consts = pools["consts"]

    # ---- constants ----
    ident = consts.tile([128, 128], F32)
    make_identity(nc, ident)
    eye8 = consts.tile([128, 128], FP8)
    nc.scalar.activation(eye8, ident, AF.Copy)
    ones_f3 = consts.tile([128, 2, 128], F32)
    nc.vector.memset(ones_f3, 1.0)
    ones8 = consts.tile([128, 2, 128], FP8)
    nc.scalar.activation(ones8, ones_f3, AF.Copy)
    s2_col = consts.tile([128, 1], F32)
    nc.vector.memset(s2_col, S2)

    # Warm-up matmuls: PE sits idle until the first strip lands; HAM then
    # runs the first real matmul bursts at the throttled clock. Burn the
    # idle head on dependency-free matmuls to hold K=8/8.
    wu = ps_mm2.tile([128, 512], F32, tag="mm2")
    for i in range(40):
        nc.tensor.matmul(wu[:, 0:128], ones8[:, 0, :], eye8,
                         start=(i == 0), stop=(i == 39))
    wusb = consts.tile([128, 1], F32)
    nc.scalar.activation(wusb, wu[:, 0:1], AF.Copy)

    # ---- small persistent tensors ----
    wt_t = []
    for ft in range(2):
        w = consts.tile([128, F], BF16, tag=f"wt{ft}")
        nc.sync.dma_start(out=w, in_=wtdram[128 * ft:128 * (ft + 1), :])
        wt_t.append(w)
    brow = consts.tile([1, F], F32)
    nc.scalar.dma_start(out=brow, in_=bdram[:, :])
    b_col = []
    for ob in range(2):
        pb = ps_small.tile([128, 1], F32, tag="small")
        nc.tensor.transpose(pb, brow[:, 128 * ob:128 * (ob + 1)], ident[0:1, 0:1])
        bc = consts.tile([128, 1], F32, tag=f"bcol{ob}")
        nc.scalar.activation(bc, pb, AF.Copy)
        b_col.append(bc)

    strips = [None] * NSTRIP
    rep_d = [None] * NSTRIP   # (1/S) * d_i per strip, replicated across parts
    xph = [consts.tile([128, 2, F], FP8, tag=f"xph{m}", name=f"xph{m}")
           for m in range(NPAIR)]
    xpl = ([consts.tile([128, 2, F], FP8, tag=f"xpl{m}", name=f"xpl{m}")
            for m in range(NPAIR)] if XSPLIT else None)
    acc = {}
    hT = {}
    seg_done = {}

    def emit_dmas(c):
        # Emit every input DMA up front so the two HWDGE rings and the
        # SWDGE queue drain back-to-back. Each strip is split across both
        # rings so strips complete strictly one after another, unlocking
        # rowsums/matmul work early.
        st = sb.tile([128, NT, SW], FP8, tag="strip", bufs=NSTRIP)
        strips[c] = st
        a3 = adjs[c].rearrange("p (t j) -> p t j", t=NT)
        nc.sync.dma_start(out=st[:, 0:NT // 2, :], in_=a3[:, 0:NT // 2, :])
        nc.scalar.dma_start(out=st[:, NT // 2:, :], in_=a3[:, NT // 2:, :])
        # X rows for the k-blocks whose d comes from this strip: one
        # contiguous SWDGE transfer that stays off the two HWDGE rings.
        xc = sb.tile([128, 4, F], BF16, tag="x", bufs=NSTRIP)
        nc.gpsimd.dma_start(out=xc, in_=xdram[c].rearrange(
            "p (h f) -> p h f", h=4))
        return xc

    def emit_strip(c, xc):
        st = strips[c]
        # rowsum of adj rows [512c, 512c+512): all-ones.T @ strip blocks,
        # 8 DoubleRow matmuls accumulated in PSUM; every psum partition
        # holds the same column-sums, so d comes out already replicated.
        pr = ps_small.tile([128, SW], F32, tag="small")
        for m in range(NPAIR):
            nc.tensor.matmul(pr, ones8, st[:, 2 * m:2 * m + 2, :],
                             start=(m == 0), stop=(m == NPAIR - 1),
                             perf_mode=DR)
        # rep_d = 1 / sqrt(S^2 r + S^2) = (1/S) / sqrt(r + 1): compute on a
        # single [1, 512] row (pr rows are identical), then let the idle
        # gpsimd engine broadcast it across partitions — avoids the wide
        # DVE reciprocal that dominated the vector engine before.
        srow = sb.tile([1, SW], F32, tag="srow", bufs=2)
        nc.scalar.activation(srow, pr[0:1, :], AF.Sqrt,
                             bias=s2_col[0:1, :], scale=s2_col[0:1, :])
        rrow = sb.tile([1, SW], F32, tag="rrow", bufs=2)
        nc.vector.reciprocal(rrow, srow)
        rd = sb.tile([128, SW], F32, tag="repd", bufs=NSTRIP)
        nc.gpsimd.partition_broadcast(rd, rrow)
        rep_d[c] = rd
        for h in range(4):
            t = 4 * c + h
            # transpose one [1,128] slice of rrow into column form, then
            # scale by S^2 to get S*d_k for the X' row scaling.
            tcol = ps_small.tile([128, 1], F32, tag="small")
            nc.tensor.transpose(tcol, rrow[:, 128 * h:128 * (h + 1)],
                                ident[0:1, 0:1])
            dcolS = sb.tile([128, 1], F32, tag="dcol", bufs=8)
            nc.scalar.activation(dcolS, tcol, AF.Copy, scale=s2_col)
            t1 = sb.tile([128, F], F32, tag="t1", bufs=4)
            nc.scalar.activation(t1, xc[:, h, :], AF.Copy, scale=dcolS)
            m, j = t // 2, t % 2
            nc.scalar.activation(xph[m][:, j, :], t1, AF.Copy)
            if XSPLIT:
                nc.vector.tensor_sub(xpl[m][:, j, :], t1, xph[m][:, j, :])

    def emit_seg(fb, ic, s):
        """Segment s (k-pairs 2s, 2s+1 = strip s) of H^T chunk (fb, ic)."""
        P = ps_mm1.tile([128, 512], F32, tag="mm1")
        fsl = slice(128 * fb, 128 * (fb + 1))
        n_mm = 2 * (2 if XSPLIT else 1) + (4 if s == ic else 0)
        i_mm = 0
        for p in (2 * s, 2 * s + 1):
            rhs = strips[ic][:, 2 * p:2 * p + 2, :]
            nc.tensor.matmul(P, xph[p][:, :, fsl], rhs,
                             start=(i_mm == 0), stop=(i_mm == n_mm - 1),
                             perf_mode=DR)
            i_mm += 1
            if XSPLIT:
                nc.tensor.matmul(P, xpl[p][:, :, fsl], rhs,
                                 start=False, stop=(i_mm == n_mm - 1),
                                 perf_mode=DR)
                i_mm += 1
        if s == ic:
            # identity fold: adds S*(d*X)^T into columns 128j..128j+128
            for jj in range(4):
                t = 4 * ic + jj
                nc.tensor.matmul(
                    P[:, 128 * jj:128 * (jj + 1)],
                    xph[t // 2][:, t % 2, fsl], eye8,
                    start=False, stop=(i_mm == n_mm - 1))
                i_mm += 1
        key = (fb, ic)
        ns = seg_done.get(key, 0)
        if ns == 0:
            a = sb.tile([128, 512], F32, tag="acc", bufs=8)
            nc.scalar.activation(a, P, AF.Copy)
            acc[key] = a
        else:
            nc.vector.tensor_add(acc[key], acc[key], P)
        seg_done[key] = ns + 1
        if seg_done[key] == NSTRIP:
            h = sb.tile([128, 512], BF16, tag="hT", bufs=4)
            nc.vector.tensor_mul(h, acc[key], rep_d[ic])
            hT[key] = h

    def emit_mm2(ic):
        for ob in range(2):
            P2 = ps_mm2.tile([128, 512], F32, tag="mm2")
            for fb in range(2):
                nc.tensor.matmul(
                    P2, wt_t[fb][:, 128 * ob:128 * (ob + 1)], hT[(fb, ic)],
                    start=(fb == 0), stop=(fb == 1))
            o = sb.tile([128, 512], F32, tag="osb", bufs=4)
            nc.scalar.activation(o, P2, AF.Relu, bias=b_col[ob])
            nc.sync.dma_start(
                out=outT[128 * ob:128 * (ob + 1), 512 * ic:512 * (ic + 1)],
                in_=o)

    xcs = [emit_dmas(c) for c in range(NSTRIP)]
    for c in range(NSTRIP):
        emit_strip(c, xcs[c])
        # seg (fb, ic, s) is ready once strips max(ic, s) .. 0 arrived;
        # chunks with ic < c complete first so their mm2 overlaps the
        # remaining segments of later chunks.
        for ic in range(c + 1):
            for s in (range(c + 1) if ic == c else [c]):
                for fb in range(2):
                    emit_seg(fb, ic, s)
            if (0, ic) in hT and (1, ic) in hT:
                emit_mm2(ic)


_CACHE = {}


def _build():
    if "nc" in _CACHE:
        return _CACHE["nc"]
    _install_axon_hooks()
    nc = bacc.Bacc("TRN2", target_bir_lowering=False, debug=False,
                   num_devices=N_CORES)
    adjs = [nc.dram_tensor(f"adjs{c}", [128, NT * SW], FP8,
                           kind="ExternalInput").ap()
            for c in range(NSTRIP)]
    xdram = [nc.dram_tensor(f"x{c}", [128, 4 * F], BF16,
                            kind="ExternalInput").ap()
             for c in range(NSTRIP)]
    wtdram = nc.dram_tensor("wt", [F, F], BF16, kind="ExternalInput").ap()
    bdram = nc.dram_tensor("b", [1, F], F32, kind="ExternalInput").ap()
    outT = nc.dram_tensor("outT", [F, N], F32, kind="ExternalOutput").ap()

    with tile.TileContext(nc) as tc:
        with tc.tile_pool(name="consts", bufs=1) as consts, \
             tc.tile_pool(name="sb", bufs=2) as sb, \
             tc.tile_pool(name="ps_mm1", bufs=3, space="PSUM") as ps_mm1, \
             tc.tile_pool(name="ps_small", bufs=3, space="PSUM") as ps_small, \
             tc.tile_pool(name="ps_mm2", bufs=2, space="PSUM") as ps_mm2:
            pools = dict(consts=consts, sb=sb, ps_mm1=ps_mm1,
                         ps_small=ps_small, ps_mm2=ps_mm2)
            _emit(nc, tc, pools, adjs, xdram, wtdram, bdram, outT)
    nc.compile()
    _CACHE["nc"] = nc
    return nc


def _shard(inputs):
    X = np.ascontiguousarray(np.asarray(inputs["X"], dtype=np.float32))
    adj = np.asarray(inputs["adj"], dtype=np.float32)
    W = np.asarray(inputs["W"], dtype=np.float32)
    b = np.asarray(inputs["b"], dtype=np.float32)
    np_fp8 = mybir.dt.np(FP8)
    np_bf16 = mybir.dt.np(BF16)
    wt = np.ascontiguousarray(W.T).astype(np_bf16)
    brow = b.reshape(1, F)
    in_maps = []
    for c in range(N_CORES):
        # adjs[s][p, t*SW + jj] = adj[c][512 s + jj, 128 t + p]
        a4 = adj[c].reshape(NSTRIP, SW, NT, 128)
        strips = np.ascontiguousarray(a4.transpose(0, 3, 2, 1)).reshape(
            NSTRIP, 128, NT * SW).astype(np_fp8)
        m = {f"adjs{s}": strips[s] for s in range(NSTRIP)}
        xs = X[c].reshape(NSTRIP, 4, 128, F).transpose(0, 2, 1, 3)
        xs = np.ascontiguousarray(xs).reshape(NSTRIP, 128, 4 * F).astype(np_bf16)
        for s in range(NSTRIP):
            m[f"x{s}"] = xs[s]
        m["wt"] = wt
        m["b"] = brow
        in_maps.append(m)
    return in_maps


def run(inputs, trace=False):
    nc = _build()
    in_maps = _shard(inputs)
    res = run_bass_kernel_spmd(
        nc, in_maps, core_ids=list(range(N_CORES)), trace=trace)
    out = np.stack([r["outT"].T for r in res.results]).astype(np.float32)
    return np.ascontiguousarray(out), res


def kernel(**inputs):
    return run(inputs, trace=False)[0]
